# revision 1
# baseline (speedup 1.0000x reference)
"""Multi-head causal self-attention on 8 Trainium2 NeuronCores.

Problem: nn_MultiHeadSelfAttention (B=2, T=2048, D=1024, H=16, d_k=64), f32.
Returns (out [B,T,D], weights [B,H,T,T]) like the reference.

Sharding (tensor-parallel over heads, data-parallel over batch):
  core c in 0..7 handles batch b=c//4 and heads h0=(c%4)*4 .. h0+3.
  Wq/Wk/Wv are split column-wise (head slices), Wo row-wise; each core
  emits its 4 heads' softmax weights and a partial output projection;
  the host sums the 4 partials per batch and adds bo.

Kernel strategy (per core):
  - QT/KT [256,2048] head-dim-major and V [2048,256] computed with f32r
    matmuls (full PE rate, ~1e-4 relative precision, vs 1/4 rate for f32).
  - Per head: scores computed twice, in [s,t] layout (for A@V, with a
    fused ones-column giving softmax denominators Z in the same matmul)
    and in [t,s] layout where exp(S - lnZ) directly yields normalized
    softmax weights (Z known from the first pass; bias-folded into the
    ACT exp). Only lower-triangular (causal) blocks are computed; the
    upper triangle relies on zero-initialized output buffers.
"""
import numpy as np
import concourse.bass as bass
import concourse.mybir as mybir
import concourse.tile as tile
from concourse import bacc
from concourse.bass_utils import run_bass_kernel_spmd

F32 = mybir.dt.float32
FR = mybir.dt.float32r
AF = mybir.ActivationFunctionType

B, T, D, H, DK = 2, 2048, 1024, 16, 64
HPC = 4            # heads per core
CD = HPC * DK      # local head-dim total = 256
NT = T // 128      # 16 t-tiles
NCH = T // 512     # 4 t-chunks


def R(ap):
    return ap.bitcast(FR)


def build_nc(n_tt=NT):
    nc = bacc.Bacc("TRN2", target_bir_lowering=False, debug=False, num_devices=8)

    xT = nc.dram_tensor("xT", [D, T], FR, kind="ExternalInput")
    wqT = nc.dram_tensor("wqT", [D, CD], FR, kind="ExternalInput")
    wkT = nc.dram_tensor("wkT", [D, CD], FR, kind="ExternalInput")
    wvT = nc.dram_tensor("wvT", [D, CD], FR, kind="ExternalInput")
    woT = nc.dram_tensor("woT", [CD, D], FR, kind="ExternalInput")
    bq = nc.dram_tensor("bq", [2, 128], F32, kind="ExternalInput")  # pre-scaled 1/8
    bk = nc.dram_tensor("bk", [2, 128], F32, kind="ExternalInput")
    bv = nc.dram_tensor("bv", [1, CD], FR, kind="ExternalInput")
    tri = nc.dram_tensor("tri", [128, 128], F32, kind="ExternalInput")
    mb2 = nc.dram_tensor("mb2", [4, 128, 512], FR, kind="ExternalInput")
    idm = nc.dram_tensor("idm", [128, 128], F32, kind="ExternalInput")
    onesv = nc.dram_tensor("onesv", [128, 128], FR, kind="ExternalInput")

    out_w = nc.dram_tensor("out_w", [HPC, T, T], F32, kind="ExternalOutput")
    out_p = nc.dram_tensor("out_p", [T, D], F32, kind="ExternalOutput")

    nch = (n_tt + 3) // 4

    with tile.TileContext(nc) as tc:
        with tc.tile_pool(name="persist", bufs=1) as P, \
             tc.tile_pool(name="stats", bufs=2) as SP, \
             tc.tile_pool(name="epool", bufs=4) as EP, \
             tc.tile_pool(name="wpool", bufs=4) as WP, \
             tc.tile_pool(name="opool", bufs=2) as OP, \
             tc.tile_pool(name="psA", bufs=4, space="PSUM") as psA, \
             tc.tile_pool(name="psAV", bufs=2, space="PSUM") as psAV:

            wq_s = P.tile([128, 8 * CD], FR, tag="wq_s")
            wk_s = P.tile([128, 8 * CD], FR, tag="wk_s")
            wv_s = P.tile([128, 8 * CD], FR, tag="wv_s")
            wo_s = P.tile([128, 2 * D], FR, tag="wo_s")
            bq_s = P.tile([128, 2], F32, tag="bq_s")
            bk_s = P.tile([128, 2], F32, tag="bk_s")
            bv_s = P.tile([1, CD], FR, tag="bv_s")
            tri_s = P.tile([128, 128], F32, tag="tri_s")
            mb2_s = P.tile([128, 4 * 512], FR, tag="mb2_s")
            id_s = P.tile([128, 128], F32, tag="id_s")
            one128 = P.tile([1, 128], FR, tag="one128")
            one64 = P.tile([1, 64], FR, tag="one64")
            qt = [P.tile([128, T], FR, tag=f"qt{m}", name=f"qt{m}") for m in range(2)]
            kt = [P.tile([128, T], FR, tag=f"kt{m}", name=f"kt{m}") for m in range(2)]
            v_s = P.tile([128, NT * 260], FR, tag="v_s")
            hc = [P.tile([128, T], FR, tag=f"hc{m}", name=f"hc{m}") for m in range(2)]

            dma = nc.sync.dma_start
            act = nc.scalar.activation
            vec = nc.vector

            dma(out=wq_s.rearrange("p (k c) -> p k c", k=8),
                in_=wqT.rearrange("(k p) c -> p k c", p=128))
            dma(out=wk_s.rearrange("p (k c) -> p k c", k=8),
                in_=wkT.rearrange("(k p) c -> p k c", p=128))
            dma(out=wv_s.rearrange("p (k c) -> p k c", k=8),
                in_=wvT.rearrange("(k p) c -> p k c", p=128))
            dma(out=wo_s.rearrange("p (k c) -> p k c", k=2),
                in_=woT.rearrange("(k p) c -> p k c", p=128))
            dma(out=bq_s, in_=bq.rearrange("m p -> p m"))
            dma(out=bk_s, in_=bk.rearrange("m p -> p m"))
            dma(out=bv_s, in_=bv[:, :])
            dma(out=tri_s, in_=tri[:, :])
            dma(out=mb2_s.rearrange("p (v c) -> p v c", v=4),
                in_=mb2.rearrange("v p c -> p v c"))
            dma(out=id_s, in_=idm[:, :])
            dma(out=one128, in_=onesv[0:1, :])
            dma(out=one64, in_=onesv[0:1, 0:64])
            dma(out=v_s.rearrange("p (blk h e) -> p blk h e", blk=NT, h=HPC)[:, :, :, 64:65],
                in_=onesv[:, 0:64].rearrange("p (a b) -> p a b", a=NT))

            # ---------- phase A: projections ----------
            with tc.tile_pool(name="xk", bufs=1) as XK:
                xk = [XK.tile([128, T], FR, tag=f"xk{k}", name=f"xk{k}")
                      for k in range(8)]
                for k in range(8):
                    dma(out=xk[k], in_=xT[128 * k:128 * (k + 1), :])

                for (wt, qkt, bias, scale) in ((wq_s, qt, bq_s, 0.125),
                                               (wk_s, kt, bk_s, 1.0)):
                    for m in range(2):
                        for tch in range(NCH):
                            ps = psA.tile([128, 512], F32, tag="psA")
                            for k in range(8):
                                nc.tensor.matmul(
                                    ps,
                                    R(wt[:, 256 * k + 128 * m: 256 * k + 128 * m + 128]),
                                    R(xk[k][:, 512 * tch: 512 * (tch + 1)]),
                                    start=(k == 0), stop=(k == 7))
                            act(qkt[m][:, 512 * tch: 512 * (tch + 1)], ps,
                                AF.Identity, bias=bias[:, m:m + 1], scale=scale)

                for blk in range(NT):
                    ps = psA.tile([128, 512], F32, tag="psA")
                    for k in range(8):
                        nc.tensor.matmul(
                            ps[:, 0:CD],
                            R(xk[k][:, 128 * blk: 128 * (blk + 1)]),
                            R(wv_s[:, 256 * k: 256 * (k + 1)]),
                            start=(k == 0), stop=False)
                    nc.tensor.matmul(ps[:, 0:CD], R(one128), R(bv_s),
                                     start=False, stop=True)
                    vec.tensor_copy(
                        v_s[:, 260 * blk: 260 * blk + 260]
                        .rearrange("p (h e) -> p h e", h=HPC)[:, :, 0:64],
                        ps[:, 0:CD].rearrange("p (h e) -> p h e", e=64))

            # ---------- phase B: attention per head ----------
            for h in range(HPC):
                m, po = h // 2, 64 * (h % 2)
                qh = qt[m][po:po + 64, :]
                kh = kt[m][po:po + 64, :]

                r_all = SP.tile([128, 16], F32, tag="r_all")
                lnr = SP.tile([128, 16], F32, tag="lnr")

                # B2: E' in [s,t]; AV + Z fused via ones column
                for c in range(nch):
                    nblk = 4 * c + 4
                    av = psAV.tile([65, 512], F32, tag="psAV")
                    es = []
                    for blk in range(nblk):
                        sp = psA.tile([128, 512], F32, tag="psA")
                        nc.tensor.matmul(
                            sp,
                            R(kh[:, 128 * blk: 128 * (blk + 1)]),
                            R(qh[:, 512 * c: 512 * (c + 1)]),
                            start=True, stop=True)
                        e = EP.tile([128, 512], FR, tag="e")
                        act(e, sp, AF.Exp)
                        if blk >= 4 * c:
                            r_ = blk - 4 * c
                            vec.tensor_mul(e, e, mb2_s[:, 512 * r_: 512 * (r_ + 1)])
                        es.append(e)
                        if blk > 0:
                            pb = blk - 1
                            nc.tensor.matmul(
                                av,
                                R(v_s[:, 260 * pb + 65 * h: 260 * pb + 65 * h + 65]),
                                R(es[pb]),
                                start=(pb == 0), stop=False)
                    pb = nblk - 1
                    nc.tensor.matmul(
                        av,
                        R(v_s[:, 260 * pb + 65 * h: 260 * pb + 65 * h + 65]),
                        R(es[pb]),
                        start=(pb == 0), stop=True)

                    # Z -> per-t-tile reciprocal columns (tiny PE transposes)
                    ztmp = SP.tile([1, 512], F32, tag="ztmp")
                    vec.tensor_copy(ztmp, av[64:65, :])
                    for k in range(4):
                        i_tt = 4 * c + k
                        zt = psA.tile([128, 4], F32, tag="psA")
                        nc.tensor.matmul(zt[:, 0:1], ztmp[0:1, 128 * k:128 * (k + 1)],
                                         id_s[0:1, 0:1], is_transpose=True,
                                         start=True, stop=True)
                        vec.reciprocal(r_all[:, i_tt:i_tt + 1], zt[:, 0:1])
                    rrow = SP.tile([1, 512], FR, tag="rrow")
                    with nc.allow_low_precision(reason="f32r is f32-width"):
                        vec.reciprocal(rrow, av[64:65, :])
                    rb_ps = psA.tile([64, 512], F32, tag="psA")
                    nc.tensor.matmul(rb_ps, R(one64), R(rrow), start=True, stop=True)
                    rbc = SP.tile([64, 512], F32, tag="rbc")
                    vec.tensor_copy(rbc, rb_ps)
                    vec.tensor_mul(
                        hc[m][po:po + 64, 512 * c: 512 * (c + 1)],
                        av[0:64, :], rbc)

                act(lnr, r_all, AF.Ln)

                # B1: normalized weights in [t,s] layout
                for i in range(n_tt):
                    jmax = (i + 4) // 4
                    rem = (i + 1) * 128 - 512 * (jmax - 1)
                    for j in range(jmax):
                        N = 512 if j < jmax - 1 else rem
                        sp = psA.tile([128, 512], F32, tag="psA")
                        nc.tensor.matmul(
                            sp[:, 0:N],
                            R(qh[:, 128 * i: 128 * (i + 1)]),
                            R(kh[:, 512 * j: 512 * j + N]),
                            start=True, stop=True)
                        w_sb = WP.tile([128, 512], F32, tag="w")
                        act(w_sb[:, 0:N], sp[:, 0:N], AF.Exp,
                            bias=lnr[:, i:i + 1], scale=1.0)
                        if j == jmax - 1:
                            vec.tensor_mul(w_sb[:, N - 128:N],
                                           w_sb[:, N - 128:N], tri_s)
                        dma(out=out_w[h, 128 * i: 128 * (i + 1), 512 * j: 512 * j + N],
                            in_=w_sb[:, 0:N])

            # ---------- output projection ----------
            for c in range(nch):
                for q in range(4):
                    tt = 4 * c + q
                    if tt >= n_tt:
                        break
                    o_sb = OP.tile([128, D], F32, tag="o")
                    for n in range(2):
                        op = psA.tile([128, 512], F32, tag="psA")
                        for k in range(2):
                            nc.tensor.matmul(
                                op,
                                R(hc[k][:, 512 * c + 128 * q: 512 * c + 128 * (q + 1)]),
                                R(wo_s[:, 1024 * k + 512 * n: 1024 * k + 512 * (n + 1)]),
                                start=(k == 0), stop=(k == 1))
                        vec.tensor_copy(o_sb[:, 512 * n: 512 * (n + 1)], op)
                    dma(out=out_p[128 * tt: 128 * (tt + 1), :], in_=o_sb)

    nc.compile()
    return nc


def make_host_inputs(x, Wq, bq, Wk, bk, Wv, bv, Wo):
    tri = np.tril(np.ones((128, 128), dtype=np.float32))
    mb2 = np.zeros((4, 128, 512), dtype=np.float32)
    for r_ in range(4):
        mb2[r_, :, 128 * r_:128 * (r_ + 1)] = tri.T
        mb2[r_, :, 128 * (r_ + 1):] = 1.0
    idm = np.eye(128, dtype=np.float32)

    in_maps = []
    for c in range(8):
        b = c // 4
        r0 = (c % 4) * CD
        in_maps.append({
            "xT": np.ascontiguousarray(x[b].T),
            "wqT": np.ascontiguousarray(Wq[r0:r0 + CD, :].T),
            "wkT": np.ascontiguousarray(Wk[r0:r0 + CD, :].T),
            "wvT": np.ascontiguousarray(Wv[r0:r0 + CD, :].T),
            "woT": np.ascontiguousarray(Wo[:, r0:r0 + CD].T),
            "bq": (bq[r0:r0 + CD] * 0.125).reshape(2, 128).astype(np.float32),
            "bk": bk[r0:r0 + CD].reshape(2, 128).astype(np.float32),
            "bv": bv[r0:r0 + CD].reshape(1, CD).astype(np.float32),
            "tri": tri, "mb2": mb2, "idm": idm,
            "onesv": np.ones((128, 128), dtype=np.float32),
        })
    return in_maps


def assemble(results, bo):
    weights = np.empty((B, H, T, T), dtype=np.float32)
    out = np.zeros((B, T, D), dtype=np.float32)
    for c in range(8):
        b = c // 4
        h0 = (c % 4) * HPC
        weights[b, h0:h0 + HPC] = results[c]["out_w"]
        out[b] += results[c]["out_p"]
    out += bo.astype(np.float32)
    return out, weights


_CACHE = {}


def kernel(x, Wq, bq, Wk, bk, Wv, bv, Wo, bo):
    x, Wq, bq, Wk, bk, Wv, bv, Wo, bo = (
        np.asarray(a, dtype=np.float32)
        for a in (x, Wq, bq, Wk, bk, Wv, bv, Wo, bo))
    if "nc" not in _CACHE:
        _CACHE["nc"] = build_nc()
    in_maps = make_host_inputs(x, Wq, bq, Wk, bk, Wv, bv, Wo)
    res = run_bass_kernel_spmd(_CACHE["nc"], in_maps, list(range(8)))
    return assemble(res.results, bo)
